# revision 1
# baseline (speedup 1.0000x reference)
"""Trainium2 Bass kernel for NeocortexBlock: RMSNorm + per-head 4-layer GELU MLP.

reference semantics (fp32):
    h  = rms_norm(x, g)                      # over last dim (2048)
    hp = h.reshape(B, S, 16, 128)
    hp = gelu(hp @ w0) -> gelu(. @ w1) -> gelu(. @ w2) -> (. @ w3)
    out = hp.reshape(B, S, 2048)

Sharding: data-parallel over tokens. 16384 tokens / 8 cores = 2048 tokens per
core; weights replicated on every core. Each core runs the full per-head MLP
chain for its tokens. No collectives.

Per-core kernel (Tile framework):
  Phase 1: per 128-token tile: DMA x (token-major, fp32), bn_stats/bn_aggr to
    get mean&var -> mean(x^2) = var + mean^2 -> rs = 1/sqrt(+eps); scale+cast
    to bf16 on DVE; PE-transpose 128x128 tiles into feature-major xn
    [128 d, token] bf16 (d_head=128 = exactly one partition tile per head).
  Phase 2: per head h, per 512-token block: bf16 matmul chain on PE with
    fp32 PSUM accumulation (stationary = weight k-tiles [128,128], moving =
    activations [128,512]); exact-GELU on scalar engine writes bf16 SBUF;
    final [128 d, 512 t] result PE-transposed back to token-major and DMA'd
    out as fp32.

g (RMSNorm weight) is folded into w0 on the host: (x*rs*g) @ w0 = (x*rs) @
(diag(g) w0). Weights are cast to bf16 on the host.
"""

import numpy as np
import ml_dtypes

D_TOTAL, H, D_HEAD, HID = 2048, 16, 128, 512
N_CORES = 8
B, S = 4, 4096
TOKENS = B * S                  # 16384
T = TOKENS // N_CORES           # 2048 tokens per core
TT = T // 128                   # 16 token tiles per core
NB = T // 512                   # 4 token blocks per core
EPS = float(np.finfo(np.float32).eps)

_CACHE = {}


def _build():
    from contextlib import ExitStack

    import concourse.bass as bass
    import concourse.mybir as mybir
    import concourse.tile as tile
    from concourse import bacc
    from concourse.bass import ts
    from concourse.masks import make_identity

    f32 = mybir.dt.float32
    bf16 = mybir.dt.bfloat16
    GELU = mybir.ActivationFunctionType.Gelu
    SQRT = mybir.ActivationFunctionType.Sqrt

    nc = bacc.Bacc("TRN2", target_bir_lowering=False, debug=False,
                   enable_asserts=True, num_devices=N_CORES)
    x = nc.dram_tensor("x", [T, D_TOTAL], f32, kind="ExternalInput").ap()
    w0 = nc.dram_tensor("w0", [H, D_HEAD, HID], bf16, kind="ExternalInput").ap()
    w1 = nc.dram_tensor("w1", [H, HID, HID], bf16, kind="ExternalInput").ap()
    w2 = nc.dram_tensor("w2", [H, HID, HID], bf16, kind="ExternalInput").ap()
    w3 = nc.dram_tensor("w3", [H, HID, D_HEAD], bf16, kind="ExternalInput").ap()
    y = nc.dram_tensor("y", [T, D_TOTAL], f32, kind="ExternalOutput").ap()

    with ExitStack() as ctx:
        tc = ctx.enter_context(tile.TileContext(nc))
        singles = ctx.enter_context(tc.tile_pool(name="singles", bufs=1))
        xn_pool = ctx.enter_context(tc.tile_pool(name="xn_pool", bufs=1))
        stage = ctx.enter_context(tc.tile_pool(name="stage", bufs=2))
        stats = ctx.enter_context(tc.tile_pool(name="stats", bufs=4))
        wpool = ctx.enter_context(tc.tile_pool(name="wpool", bufs=2))
        gpool = ctx.enter_context(tc.tile_pool(name="gpool", bufs=2))
        opool = ctx.enter_context(tc.tile_pool(name="opool", bufs=2))
        ps_mm = ctx.enter_context(tc.tile_pool(name="ps_mm", bufs=3, space="PSUM"))
        ps_tr = ctx.enter_context(tc.tile_pool(name="ps_tr", bufs=2, space="PSUM"))

        ident_bf = singles.tile([128, 128], bf16)
        make_identity(nc, ident_bf)
        ident_f = singles.tile([128, 128], f32)
        make_identity(nc, ident_f)
        eps_t = singles.tile([128, 1], f32)
        nc.vector.memset(eps_t, EPS)

        # feature-major normed activations: [d_in_head, token_tile, head, token]
        xn = xn_pool.tile([128, TT, H, 128], bf16)

        # ---- Phase 1: RMSNorm + transpose to feature-major ----
        x_r = x.rearrange("(n p) d -> n p d", p=128)
        for tt in range(TT):
            xt = stage.tile([128, D_TOTAL], f32, tag="xt")
            nc.sync.dma_start(out=xt, in_=x_r[tt])
            st = stats.tile([128, 4, 6], f32, tag="st")
            for s in range(4):
                nc.vector.bn_stats(out=st[:, s, :], in_=xt[:, ts(s, 512)])
            mv = stats.tile([128, 2], f32, tag="mv")
            nc.vector.bn_aggr(out=mv, in_=st)
            # mean(x^2) = var + mean^2
            msq = stats.tile([128, 1], f32, tag="msq")
            nc.vector.tensor_mul(out=msq, in0=mv[:, 0:1], in1=mv[:, 0:1])
            nc.vector.tensor_add(out=msq, in0=msq, in1=mv[:, 1:2])
            rsq = stats.tile([128, 1], f32, tag="rsq")
            nc.scalar.activation(out=rsq, in_=msq, func=SQRT, bias=eps_t)
            rs = stats.tile([128, 1], f32, tag="rs")
            nc.vector.reciprocal(out=rs, in_=rsq)
            xs = stage.tile([128, D_TOTAL], bf16, tag="xs")
            nc.vector.tensor_scalar_mul(out=xs, in0=xt, scalar1=rs)
            for hq in range(4):
                pt = ps_tr.tile([128, 512], bf16, tag="tr")
                for hh in range(4):
                    h = hq * 4 + hh
                    nc.tensor.transpose(pt[:, ts(hh, 128)], xs[:, ts(h, 128)],
                                        ident_bf)
                nc.vector.tensor_copy(
                    out=xn[:, tt, hq * 4:(hq + 1) * 4, :],
                    in_=pt.rearrange("p (a b) -> p a b", a=4),
                )

        # ---- Phase 2: per-head MLP chain ----
        for h in range(H):
            w0t = wpool.tile([128, HID], bf16, tag="w0")
            nc.sync.dma_start(out=w0t, in_=w0[h])
            w1t = wpool.tile([128, 4, HID], bf16, tag="w1")
            nc.sync.dma_start(out=w1t,
                              in_=w1[h].rearrange("(ko ki) e -> ki ko e", ki=128))
            w2t = wpool.tile([128, 4, HID], bf16, tag="w2")
            nc.sync.dma_start(out=w2t,
                              in_=w2[h].rearrange("(ko ki) e -> ki ko e", ki=128))
            w3t = wpool.tile([128, 4, D_HEAD], bf16, tag="w3")
            nc.sync.dma_start(out=w3t,
                              in_=w3[h].rearrange("(ko ki) e -> ki ko e", ki=128))
            for b in range(NB):
                rhs_x = xn[:, b * 4:(b + 1) * 4, h, :]
                # stage 0: [128 d] -> [512 e], K=128
                g0 = gpool.tile([128, 4, 512], bf16, tag="g0")
                for q in range(2):
                    ps = ps_mm.tile([128, 1024], f32, tag="mm")
                    for jj in range(2):
                        j = q * 2 + jj
                        nc.tensor.matmul(ps[:, ts(jj, 512)], w0t[:, ts(j, 128)],
                                         rhs_x, start=True, stop=True)
                    nc.scalar.activation(out=g0[:, q * 2:(q + 1) * 2, :],
                                         in_=ps.rearrange("p (a b) -> p a b", a=2),
                                         func=GELU)
                # stages 1,2: [512] -> [512], K=512 (4 k-tiles)
                gin = g0
                for wt in (w1t, w2t):
                    gout = gpool.tile([128, 4, 512], bf16,
                                      tag="g1" if wt is w1t else "g2")
                    for q in range(2):
                        ps = ps_mm.tile([128, 1024], f32, tag="mm")
                        for jj in range(2):
                            j = q * 2 + jj
                            for k in range(4):
                                nc.tensor.matmul(ps[:, ts(jj, 512)],
                                                 wt[:, k, ts(j, 128)],
                                                 gin[:, k, :],
                                                 start=(k == 0), stop=(k == 3))
                        nc.scalar.activation(out=gout[:, q * 2:(q + 1) * 2, :],
                                             in_=ps.rearrange("p (a b) -> p a b",
                                                              a=2),
                                             func=GELU)
                    gin = gout
                # stage 3: [512] -> [128 d], K=512, no gelu
                ps3 = ps_mm.tile([128, 1024], f32, tag="mm")
                for k in range(4):
                    nc.tensor.matmul(ps3[:, 0:512], w3t[:, k, :], gin[:, k, :],
                                     start=(k == 0), stop=(k == 3))
                y3sb = opool.tile([128, 512], f32, tag="y3")
                nc.vector.tensor_copy(out=y3sb, in_=ps3[:, 0:512])
                # transpose back to token-major and store
                ptT = ps_tr.tile([128, 512], f32, tag="tr")
                for j in range(4):
                    nc.tensor.transpose(ptT[:, ts(j, 128)], y3sb[:, ts(j, 128)],
                                        ident_f)
                osb = opool.tile([128, 4, 128], f32, tag="osb")
                nc.vector.tensor_copy(out=osb,
                                      in_=ptT.rearrange("p (a b) -> p a b", a=4))
                out_ap = y[b * 512:(b + 1) * 512,
                           h * 128:(h + 1) * 128].rearrange("(j p) d -> p j d",
                                                            p=128)
                nc.sync.dma_start(out=out_ap, in_=osb)

    nc.compile()
    return nc


def _get_program():
    if "nc" not in _CACHE:
        _CACHE["nc"] = _build()
    return _CACHE["nc"]


LAST_RESULTS = None


def kernel(x, g, w0, w1, w2, w3):
    global LAST_RESULTS
    from concourse.bass_utils import run_bass_kernel_spmd

    nc = _get_program()

    xf = np.ascontiguousarray(np.asarray(x, np.float32).reshape(TOKENS, D_TOTAL))
    gf = np.asarray(g, np.float32).reshape(H, D_HEAD)
    bf = ml_dtypes.bfloat16
    w0b = np.ascontiguousarray(
        (np.asarray(w0, np.float32) * gf[:, :, None]).astype(bf))
    w1b = np.ascontiguousarray(np.asarray(w1, np.float32).astype(bf))
    w2b = np.ascontiguousarray(np.asarray(w2, np.float32).astype(bf))
    w3b = np.ascontiguousarray(np.asarray(w3, np.float32).astype(bf))

    in_maps = [
        {"x": xf[c * T:(c + 1) * T], "w0": w0b, "w1": w1b, "w2": w2b, "w3": w3b}
        for c in range(N_CORES)
    ]
    res = run_bass_kernel_spmd(nc, in_maps, core_ids=list(range(N_CORES)))
    LAST_RESULTS = res
    out = np.concatenate([res.results[c]["y"] for c in range(N_CORES)], axis=0)
    return out.reshape(B, S, D_TOTAL)
